# revision 3
# baseline (speedup 1.0000x reference)
# Bass/Trainium2 kernel for nn_LoRARouter (topk_masking).
#
# Reference computes:
#   gated  = pooled @ Wg^T            [B, D]   (B=8192, D=4096)
#   logits = gated  @ Wr^T            [B, 7]
#   probs  = softmax(logits)
#   ranks  = argsort(argsort(-rand_noise))    per [7, B, :8] group
#   out[m,b,e] = probs[b,m] > 0.5 ? (rank<2)/2 : (rank<1)/1
#
# `gated` is only ever consumed by the second matmul, so
#   logits = pooled @ (Wr @ Wg)^T
# which removes the 275-GFLOP [B,D]x[D,D] matmul entirely. The problem is
# then HBM-bound: read pooled (134 MB) + Wg (67 MB, once across the fleet).
#
# Sharding (8 cores):
#   - pooled_hidden, rand_noise, output: batch-sharded (1024 rows/core)
#   - Wg: row-sharded (512 contraction rows/core); each core computes a
#     partial WeffT = (Wr@Wg)^T [4096, 7] from its shard, AllReduce(add)
#     over the 8 cores (114 KB payload) yields the full WeffT everywhere.
#   - host pre-transposes pooled to d-major ([4096, 1024] per core) so the
#     contraction dim lands on SBUF partitions with fully-contiguous DMAs.

import numpy as np

import concourse.bass as bass
import concourse.bacc as bacc
import concourse.mybir as mybir
import concourse.tile as tile
from concourse.bass_utils import run_bass_kernel_spmd

F32 = mybir.dt.float32
N_CORES = 8
B, D, NM, NE = 8192, 4096, 7, 8      # batch, d_model, n_modules, n_experts
BS = B // N_CORES                    # 1024 batch rows per core
ES = D // N_CORES                    # 512 Wg rows (contraction shard) per core
NBC = BS // 128                      # 8 batch chunks of 128 per core
NK = D // 128                        # 32 contraction chunks of 128
GRP = NM * NE                        # 56 columns per batch chunk (m*8+e)
W = NBC * GRP                        # 448 free columns in the [128, 448] tiles

ALU = mybir.AluOpType
AF = mybir.ActivationFunctionType

_CACHE = {}
LAST_RESULTS = None  # test harness introspection


def _build_program():
    nc = bacc.Bacc(
        "TRN2", target_bir_lowering=False, debug=False, num_devices=N_CORES
    )

    xT = nc.dram_tensor("xT", [D, BS], F32, kind="ExternalInput")
    wg = nc.dram_tensor("wg", [ES, D], F32, kind="ExternalInput")
    wrt = nc.dram_tensor("wrt", [128, 4 * NM], F32, kind="ExternalInput")
    nzin = nc.dram_tensor("nz", [128, W], F32, kind="ExternalInput")
    cst = nc.dram_tensor("cst", [128, W], F32, kind="ExternalInput")
    o = nc.dram_tensor("o", [128, W], F32, kind="ExternalOutput")

    # AllReduce bounce buffers ([128, 224] image of WeffT in k-chunk-major
    # SBUF layout: [p, k*7+m] == WeffT[128k+p, m]).
    weff_in = nc.dram_tensor("weff_in", [128, NK * NM], F32)
    weff_out = nc.dram_tensor("weff_out", [128, NK * NM], F32, addr_space="Shared")

    with tile.TileContext(nc) as tc:
        with (
            tc.tile_pool(name="big", bufs=1) as bp,
            tc.tile_pool(name="small", bufs=1) as sp,
            tc.tile_pool(name="scr", bufs=2) as scp,
            tc.tile_pool(name="sm", bufs=16) as smp,
            tc.tile_pool(name="ps", bufs=8, space="PSUM") as ps,
        ):
            # ---- input DMAs (nc.sync = HWDGE ring, FIFO per engine:
            # emission order is completion-priority order) ----
            wrt_sb = sp.tile([128, 4 * NM], F32, tag="wrt")
            nz = sp.tile([128, W], F32, tag="nz")
            cstt = sp.tile([128, W], F32, tag="cst")
            nc.sync.dma_start(wrt_sb[:], wrt[:])
            nc.sync.dma_start(nz[:], nzin[:])
            nc.sync.dma_start(cstt[:], cst[:])

            # Wg shard as 16 tiles (c: 4 e-chunks of 128 rows, g: 4 d-groups
            # of 1024 cols), streamed ahead of the xT tiles.
            wg_r = wg[:].rearrange("(c p) d -> c p d", p=128)
            wgt = {}
            for g in range(4):
                for c in range(4):
                    wgtile = bp.tile([128, 1024], F32, tag="wg", bufs=8)
                    nc.sync.dma_start(wgtile[:], wg_r[c, :, g * 1024:(g + 1) * 1024])
                    wgt[(c, g)] = wgtile

            # pooled^T shard, fully resident (16.8 MB of 28 MB SBUF) so the
            # DMA stream never stalls behind the collective.
            xT_r = xT[:].rearrange("(k p) b -> k p b", p=128)
            xts = []
            for k in range(NK):
                xtile = bp.tile([128, BS], F32, tag="x", bufs=NK)
                nc.sync.dma_start(xtile[:], xT_r[k])
                xts.append(xtile)

            # ---- partial WeffT = (Wr @ Wg_shard)^T, d-major in PSUM ----
            # out[128d, 7m] per (g, dl); contraction over the 512 shard rows
            # in 4 chunks of 128 (c).
            weff_acc = sp.tile([128, NK * NM], F32, tag="weffacc")
            for g in range(4):
                pst = []
                for dl in range(8):
                    pw = ps.tile([128, NM], F32, tag="ps")
                    pst.append(pw)
                for c in range(4):
                    for dl in range(8):
                        nc.tensor.matmul(
                            pst[dl][:],
                            wgt[(c, g)][:, dl * 128:(dl + 1) * 128],
                            wrt_sb[:, c * NM:(c + 1) * NM],
                            start=(c == 0),
                            stop=(c == 3),
                        )
                for dl in range(8):
                    k = g * 8 + dl
                    nc.vector.tensor_copy(
                        weff_acc[:, k * NM:(k + 1) * NM], pst[dl][:]
                    )

            # ---- AllReduce the partial WeffT across the 8 cores ----
            nc.scalar.dma_start(weff_in[:], weff_acc[:])
            nc.gpsimd.collective_compute(
                "AllReduce",
                ALU.add,
                replica_groups=[list(range(N_CORES))],
                ins=[weff_in[:]],
                outs=[weff_out[:]],
            )
            weffT = sp.tile([128, NK * NM], F32, tag="weffT")
            nc.scalar.dma_start(weffT[:], weff_out[:])

            # ---- expert ranks from rand_noise (independent of the matmuls;
            # overlaps the DMA/collective phase on DVE) ----
            # r[e] = #{j<e: v_j >= v_e} + #{j>e: v_j > v_e}  (stable-argsort
            # rank, ties broken toward lower index exactly as the reference).
            # acc starts at cst[e] = 7-e; for each offset o the single
            # comparison c = (v_{e-o} >= v_e) adds 1 at the A-position (e)
            # and subtracts 1 at the B-position (e-o).
            acc = sp.tile([128, W], F32, tag="acc")
            nc.vector.tensor_copy(acc[:], cstt[:])
            nz_r = nz[:].rearrange("p (c m e) -> p c m e", m=NM, e=NE)
            acc_r = acc[:].rearrange("p (c m e) -> p c m e", m=NM, e=NE)
            for off in range(1, NE):
                wdt = NE - off
                scr = scp.tile([128, NBC * NM * 7], F32, tag="scr")
                scr_v = scr[:, : NBC * NM * wdt].rearrange(
                    "p (c m e) -> p c m e", m=NM, e=wdt
                )
                nc.vector.tensor_tensor(
                    scr_v, nz_r[:, :, :, 0:wdt], nz_r[:, :, :, off:NE], ALU.is_ge
                )
                nc.vector.tensor_tensor(
                    acc_r[:, :, :, off:NE], acc_r[:, :, :, off:NE], scr_v, ALU.add
                )
                nc.vector.tensor_tensor(
                    acc_r[:, :, :, 0:wdt], acc_r[:, :, :, 0:wdt], scr_v, ALU.subtract
                )
# (acc now holds the rank r of each expert; consumed directly below)

            # ---- logits = xT^T @ WeffT per batch chunk, accumulated over
            # the 32 contraction chunks (k outer so every chunk is consumed
            # as its DMA lands; groups live in distinct PSUM banks) ----
            psl = []
            for bc in range(NBC):
                pl = ps.tile([128, NM], F32, tag="ps")
                psl.append(pl)
            for k in range(NK):
                for bc in range(NBC):
                    nc.tensor.matmul(
                        psl[bc][:],
                        xts[k][:, bc * 128:(bc + 1) * 128],
                        weffT[:, k * NM:(k + 1) * NM],
                        start=(k == 0),
                        stop=(k == NK - 1),
                    )

            # ---- softmax>0.5 condition + final select ----
            # cond = (prob_m > 0.5) = (exp_m > 0.5*sum_exp).  With
            # thr = 1+cond and val = 1-0.5*cond the reference select is
            #   out[e] = (r[e] < thr) * val
            # applied per (batch-chunk, module) with [128,1] scalar APs,
            # so no free-dim broadcast is ever needed.
            outt = sp.tile([128, W], F32, tag="outt")
            for bc in range(NBC):
                negmax = smp.tile([128, 1], F32, tag="negmax")
                ssum = smp.tile([128, 1], F32, tag="ssum")
                shalf = smp.tile([128, 1], F32, tag="shalf")
                expt = smp.tile([128, NM], F32, tag="expt")
                thr = smp.tile([128, NM], F32, tag="thr")
                val = smp.tile([128, NM], F32, tag="val")
                nc.vector.tensor_reduce(
                    negmax[:], psl[bc][:], mybir.AxisListType.X, ALU.max, negate=True
                )
                # expt = exp(logits - max), ssum = rowsum(expt)
                nc.scalar.activation(
                    expt[:], psl[bc][:], AF.Exp, bias=negmax[:], accum_out=ssum[:]
                )
                nc.vector.tensor_scalar_mul(shalf[:], ssum[:], 0.5)
                # thr = (exp > 0.5*sum) + 1  in {1, 2}
                nc.vector.tensor_scalar(
                    out=thr[:], in0=expt[:], scalar1=shalf[:], scalar2=1.0,
                    op0=ALU.is_gt, op1=ALU.add,
                )
                # val = 1.5 - 0.5*thr  in {1, 0.5}
                nc.vector.tensor_scalar(
                    out=val[:], in0=thr[:], scalar1=-0.5, scalar2=1.5,
                    op0=ALU.mult, op1=ALU.add,
                )
                for m in range(NM):
                    sl = slice(bc * GRP + m * NE, bc * GRP + (m + 1) * NE)
                    eng = nc.vector if (m % 2 == 0) else nc.gpsimd
                    eng.tensor_scalar(
                        out=outt[:, sl], in0=acc[:, sl],
                        scalar1=thr[:, m:m + 1], scalar2=val[:, m:m + 1],
                        op0=ALU.is_lt, op1=ALU.mult,
                    )
            nc.scalar.dma_start(o[:], outt[:])

    nc.compile()
    return nc


def _get_program():
    if "nc" not in _CACHE:
        _CACHE["nc"] = _build_program()
    return _CACHE["nc"]


def _const_input():
    base = (7.0 - np.arange(NE, dtype=np.float32))
    return np.ascontiguousarray(
        np.broadcast_to(np.tile(base, NBC * NM), (128, W))
    )


def kernel(pooled_hidden, Wg, Wr, rand_noise):
    global LAST_RESULTS
    ph = np.ascontiguousarray(np.asarray(pooled_hidden, dtype=np.float32))
    wg_full = np.ascontiguousarray(np.asarray(Wg, dtype=np.float32))
    wr = np.ascontiguousarray(np.asarray(Wr, dtype=np.float32))
    rn = np.ascontiguousarray(np.asarray(rand_noise, dtype=np.float32))

    nc = _get_program()
    cst = _const_input()

    in_maps = []
    for i in range(N_CORES):
        bsl = slice(i * BS, (i + 1) * BS)
        esl = slice(i * ES, (i + 1) * ES)
        xT_i = np.ascontiguousarray(ph[bsl, :].T)                  # [4096, 1024]
        wg_i = np.ascontiguousarray(wg_full[esl, :])               # [512, 4096]
        # wrt[p, c*7+m] = Wr[m, 512*i + 128*c + p]
        wrt_i = np.ascontiguousarray(
            wr[:, esl].T.reshape(4, 128, NM).transpose(1, 0, 2).reshape(128, 4 * NM)
        )
        # nz[p, c*56 + m*8 + e] = rn[m, 1024*i + 128*c + p, e]
        nz_i = np.ascontiguousarray(
            rn[:, bsl, :].transpose(1, 0, 2)
            .reshape(NBC, 128, GRP).transpose(1, 0, 2).reshape(128, W)
        )
        in_maps.append(
            {"xT": xT_i, "wg": wg_i, "wrt": wrt_i, "nz": nz_i, "cst": cst}
        )

    res = run_bass_kernel_spmd(nc, in_maps, list(range(N_CORES)))
    LAST_RESULTS = res

    out = np.empty((NM, B, NE), dtype=np.float32)
    for i, r in enumerate(res.results):
        oc = r["o"]  # [128, 448]
        out[:, i * BS:(i + 1) * BS, :] = (
            oc.reshape(128, NBC, NM, NE).transpose(2, 1, 0, 3).reshape(NM, BS, NE)
        )
    return out
